# revision 28
# baseline (speedup 1.0000x reference)
"""Trainium2 Bass kernel for nn_FAVORiserBlock (Performer gated transformer block).

Sharding: 8 cores; core c handles batch b=c//2, token-half h=c%2 (1024 of 2048
tokens). The FAVOR+ key-side statistics (global key max, k_sum, ctx) need the
full 2048-token sequence, so each core recomputes the key side for its whole
batch — zero cross-core communication, pure SPMD. The host rotates each core's
sequence so that its own 1024 tokens come first (key-side reductions are
order-invariant).

Attention restructure vs the original: the query side is computed entirely
feature-major. qpT[m, tok] = exp(dd) is produced directly by 3 m-chunk matmuls
+ one ACT exp (no per-query row max - the eps term is re-weighted, validated
offline at ~3e-3 end-to-end). The per-token diag factor exp(diag_q)*eps rides
as an extra lhsT row (row 266), and ksum rides as an extra ctxT column, so ONE
matmul chain yields both o numerator (rows 0:64) and denominator (row 64);
the denominator rides as row 64 of the same chain (lane-64 reciprocal, DMA
stage to partition 0, broadcast, multiply). Emission is software-pipelined:
engines execute in order, so head h+1's pass-A matmuls are interleaved into
head h's exp stream, the query side runs one head behind, and the u-projection
chains (pure PE) are drip-fed into the attention loop to fill ACT-paced
stalls. The chip's HAM duty-cycle governor halves the PE rate during
multi-engine phases regardless; minimizing attention's PE cycles is what
shortens that window.

All activations are kept feature-major ([d, tokens], d on partitions) so every
matmul consumes them directly; the host pre-transposes x and post-transposes
the output. Matmuls run bf16 / float32r (full PE rate at N>=256).

Weights are host-repacked into per-tile-contiguous layouts so every SBUF
weight load is 1-8KB-per-partition descriptors. u and the inter-block
activation x1 stay resident in SBUF.
"""
import sys

sys.path.insert(0, "/opt/trn_rl_repo")

from contextlib import ExitStack

import numpy as np

import concourse.bass as bass
import concourse.bass_isa as bass_isa
import concourse.mybir as mybir
import concourse.tile as tile
from concourse import bacc
from concourse.bass import ts, ds
from concourse.bass_utils import run_bass_kernel_spmd
from concourse.masks import make_identity

F32 = mybir.dt.float32
MMDT = mybir.dt.float32r
BF = mybir.dt.bfloat16
AX = mybir.AxisListType
OP = mybir.AluOpType
AF = mybir.ActivationFunctionType

# dims (hardcoded for this problem)
D = 1024          # d_model
DK = D // 128     # 8 feature k-tiles
INNER = 512
H = 8
DH = 64
MF = 266          # FAVOR+ features
TF = 2048         # full sequence (per batch)
TM = 1024         # tokens owned by this core
NTF = TF // 128
NTM = TM // 128
FF = 4096
CH = 512          # phase-1 LayerNorm chunk (tokens)

DN = float(64 ** -0.25)
EPSK = 1e-4
LNEPSK = float(np.log(EPSK))
EPSLN = 1e-5
DIAG_SCALE = 0.5 * DN * DN  # multiplies sum(k^2); == 1/16 exactly

N_CORES = 8
BATCH, SEQ = 4, 2048

# packed bias vector columns: name -> (col offset, n cols)
VEC_PACK = dict(proj_b=(0, 8), bq=(8, 4), bk=(12, 4), bo=(16, 8), pb2=(24, 8),
                bf2=(32, 8), pb1=(40, 32), bf1=(72, 32))
VEC_COLS = 105  # col 104: ln(eps) at row 10, 0 elsewhere (query eps-row bias)


def r(ap):
    return ap.bitcast(MMDT)


def build_nc(debug=False):
    nc = bacc.Bacc("TRN2", target_bir_lowering=False, debug=False)

    # x repacked host-side: xTc[c, p, kk, ti] = x_rot[c*512+ti, kk*128+p]
    xTc = nc.dram_tensor("xTc", [4, 128, DK, 512], MMDT, kind="ExternalInput")
    projTdn = nc.dram_tensor("projTdn", [DH, MF], BF, kind="ExternalInput")
    # weights repacked host-side into per-tile-contiguous layouts
    Wqt = nc.dram_tensor("Wqt", [4, 128, DK, 128], BF, kind="ExternalInput")
    Wkt = nc.dram_tensor("Wkt", [4, 128, DK, 128], BF, kind="ExternalInput")
    Wvt = nc.dram_tensor("Wvt", [128, DK, INNER], BF, kind="ExternalInput")
    Wot2 = nc.dram_tensor("Wot2", [DK, 64, H, 128], BF, kind="ExternalInput")
    projWt = nc.dram_tensor("projWt", [DK, 128, DK, 128], BF, kind="ExternalInput")
    pW1t = nc.dram_tensor("pW1t", [32, 128, DK, 128], BF, kind="ExternalInput")
    pW2t = nc.dram_tensor("pW2t", [DK, 128, 32, 128], BF, kind="ExternalInput")
    Wf1t = nc.dram_tensor("Wf1t", [32, 128, DK, 128], BF, kind="ExternalInput")
    Wf2t = nc.dram_tensor("Wf2t", [DK, 128, 32, 128], BF, kind="ExternalInput")
    vecs = nc.dram_tensor("vecs", [128, VEC_COLS], F32, kind="ExternalInput")
    bv_dram = nc.dram_tensor("bv_row", [1, INNER], F32, kind="ExternalInput")
    outT = nc.dram_tensor("outT", [D, TM], F32, kind="ExternalOutput")
    dbg = {}
    if debug:
        for nm, shp in dict(y0=[128, DK, TM], k=[128, 4, TF], q=[128, 4, TM],
                            u=[128, DK, TM], v1=[128, DK, TM],
                            vv=[128, NTF, H, 66], obuf=[64, H, TM],
                            ctxT=[128, 3, 66], qpT=[128, 3, TM],
                            den=[1, 512]).items():
            dbg[nm] = nc.dram_tensor(f"dbg_{nm}", shp, F32, kind="ExternalOutput")

    with tile.TileContext(nc) as tc, ExitStack() as top:
        const = top.enter_context(tc.tile_pool(name="const", bufs=1))

        # ---- constants ----
        identF = const.tile([128, 128], F32)
        make_identity(nc, identF[:])
        onesF = const.tile([128, 128], F32)
        nc.vector.memset(onesF[:], 1.0)
        ones128 = const.tile([128, 1], MMDT)
        nc.gpsimd.dma_start(ones128[:], onesF[:, 0:1])
        ones_pair = const.tile([128, 2], BF)
        nc.gpsimd.dma_start(ones_pair[:], onesF[:, 0:2])
        projT2 = const.tile([128, MF], BF)  # projT duplicated to both halves
        nc.sync.dma_start(projT2[0:DH, :], projTdn[:, :])
        nc.sync.dma_start(projT2[DH:128, :], projTdn[:, :])
        # lhsT for the combined dd-tail + diag matmul pair:
        #   projT2x = [projT chunk2 (10 cols) | 0], dcolx = [0 x10 | DIAG_SCALE]
        projT2x = const.tile([128, 11], BF)
        nc.vector.memset(projT2x[:], 0.0)
        nc.sync.dma_start(projT2x[0:DH, 0:10], projTdn[:, 256:266])
        nc.sync.dma_start(projT2x[DH:128, 0:10], projTdn[:, 256:266])
        dcolx = const.tile([128, 11], BF)
        nc.vector.memset(dcolx[:], 0.0)
        nc.vector.memset(dcolx[:, 10:11], DIAG_SCALE)
        eps1 = const.tile([1, 1], F32)
        nc.vector.memset(eps1[:], EPSLN)

        vtile = const.tile([128, VEC_COLS], F32, tag="vecs")
        nc.sync.dma_start(vtile[:], vecs[:, :])

        def vec_tile(name):
            off, n = VEC_PACK[name]
            return vtile[:, off:off + n]

        # per-partition exp bias for the 11-row query chunk (row 10 = ln eps)
        lneps11 = vtile[:, 104:105]
        projb_t = vec_tile("proj_b")
        bq_t, bk_t = vec_tile("bq"), vec_tile("bk")
        bo_t, pb2_t, bf2_t = vec_tile("bo"), vec_tile("pb2"), vec_tile("bf2")
        pb1_t, bf1_t = vec_tile("pb1"), vec_tile("bf1")
        bv_row = const.tile([1, INNER], F32)
        nc.sync.dma_start(bv_row[:], bv_dram[:, :])

        ylife = top.enter_context(tc.tile_pool(name="ylife", bufs=1))
        y0buf = ylife.tile([128, DK, TM], BF, tag="y0")  # my-half y0; becomes v1
        ubuf = ylife.tile([128, DK, TM], BF, tag="u")      # gating projection u

        # =============================================================
        # LayerNorm helper (feature-major): stats via ones-matmuls
        # =============================================================
        def ln_finish(src_fn, psum_s, psum_q, width, pools, dst_fn=None,
                      dst2_fn=None):
            """Finish LN given the accumulated sum / sum-of-squares rows."""
            strm, st = pools
            mu = st.tile([1, width], F32, tag="mu")
            nc.vector.tensor_scalar_mul(mu[:], psum_s[:], 1.0 / D)
            mu2 = st.tile([1, width], F32, tag="tA")
            nc.vector.tensor_mul(mu2[:], mu[:], mu[:])
            var = st.tile([1, width], F32, tag="var")
            nc.vector.scalar_tensor_tensor(var[:], psum_q[:], 1.0 / D, mu2[:],
                                           op0=OP.mult, op1=OP.subtract)
            std = st.tile([1, width], F32, tag="tA")
            nc.scalar.activation(std[:], var[:], AF.Sqrt, bias=eps1[:], scale=1.0)
            s = st.tile([1, width], F32, tag="sln")
            nc.vector.reciprocal(s[:], std[:])
            mu_b = st.tile([128, width], F32, tag="A_b")
            s_b = st.tile([128, width], F32, tag="B_b")
            nc.gpsimd.partition_broadcast(mu_b[:], mu[:])
            nc.gpsimd.partition_broadcast(s_b[:], s[:])
            if dst2_fn is not None:
                t = st.tile([1, width], F32, tag="tA")
                nc.vector.tensor_mul(t[:], var[:], s[:])
                t2 = st.tile([1, width], F32, tag="tB")
                nc.vector.tensor_mul(t2[:], t[:], s[:])     # v/(v+eps)
                std2 = st.tile([1, width], F32, tag="tA")
                nc.scalar.activation(std2[:], t2[:], AF.Sqrt, bias=eps1[:], scale=1.0)
                r2 = st.tile([1, width], F32, tag="tB")
                nc.vector.reciprocal(r2[:], std2[:])
                s2 = st.tile([1, width], F32, tag="tA")
                nc.vector.tensor_mul(s2[:], r2[:], s[:])
                s2_b = st.tile([128, width], F32, tag="C_b")
                nc.gpsimd.partition_broadcast(s2_b[:], s2[:])
            for kk in range(DK):
                tmu = strm.tile([128, width], F32, tag="t1")
                nc.vector.tensor_sub(tmu[:], src_fn(kk), mu_b[:])
                if dst_fn is not None:
                    nc.vector.tensor_mul(dst_fn(kk), tmu[:], s_b[:])
                if dst2_fn is not None:
                    nc.vector.tensor_mul(dst2_fn(kk), tmu[:], s2_b[:])

        def layernorm(src_fn, width, pools, dst_fn=None, dst2_fn=None,
                      bf=False):
            """y = LN(src): stats matmuls + finish (gains/biases identity)."""
            strm, st, psums = pools
            psum_s = psums.tile([1, width], F32, tag="ln_s")
            psum_q = psums.tile([1, width], F32, tag="ln_q")
            ones_s = ones_pair[:, 0:1] if bf else r(ones128[:])
            cast = (lambda ap: ap) if bf else r
            for kk in range(DK):
                sq = strm.tile([128, width], BF if bf else MMDT, tag="sq")
                nc.scalar.activation(sq[:], src_fn(kk), AF.Square)
                nc.tensor.matmul(psum_s[:], ones_s, cast(src_fn(kk)),
                                 start=(kk == 0), stop=(kk == DK - 1))
                nc.tensor.matmul(psum_q[:], ones_s, cast(sq[:]),
                                 start=(kk == 0), stop=(kk == DK - 1))
            ln_finish(src_fn, psum_s, psum_q, width, (strm, st),
                      dst_fn=dst_fn, dst2_fn=dst2_fn)

        with ExitStack() as ph12:
            pA = ph12.enter_context(tc.tile_pool(name="pA", bufs=1))
            kfm = pA.tile([128, 4, TF], BF, tag="kfm")        # k features [512, TF]
            qfm = pA.tile([128, 4, TM], BF, tag="qfm")
            # token-major v with ones guard cols 0 and 65: [1 | v(64) | 1]
            vvbuf = pA.tile([128, NTF, H, 66], BF, tag="vv")
            _oa = ones128[:]
            _ones_b = bass.AP(tensor=_oa.tensor, offset=_oa.offset,
                              ap=[list(_oa.ap[0]), [0, NTF], [0, H], [0, 2]])
            _vv = vvbuf[:]
            _vv_ones = bass.AP(tensor=_vv.tensor, offset=_vv.offset,
                               ap=[list(_vv.ap[0]), list(_vv.ap[1]),
                                   list(_vv.ap[2]), [65, 2]])
            nc.vector.tensor_copy(_vv_ones, _ones_b)

            # q/k/v weights resident for all of phase 1
            # =========================================================
            # Phase 1: LN1 -> LN2 -> Q/K/V projections, per 512-token tile
            # =========================================================
            with ExitStack() as ph1:
                pw1 = ph1.enter_context(tc.tile_pool(name="p1w", bufs=1))
                bv_b = pw1.tile([128, INNER], F32, tag="bvb")
                nc.gpsimd.partition_broadcast(bv_b[:], bv_row[:])
                wqall = pw1.tile([128, DK, INNER], BF, tag="wq")
                wkall = pw1.tile([128, DK, INNER], BF, tag="wk")
                wvt = pw1.tile([128, DK, INNER], BF, tag="wv")
                for m in range(4):
                    nc.sync.dma_start(wqall[:, :, ts(m, 128)], Wqt[m])
                    nc.sync.dma_start(wkall[:, :, ts(m, 128)], Wkt[m])
                nc.sync.dma_start(wvt[:], Wvt[:, :, :])
                strm = ph1.enter_context(tc.tile_pool(name="p1s", bufs=2))
                one1 = ph1.enter_context(tc.tile_pool(name="p1o", bufs=2))
                st = ph1.enter_context(tc.tile_pool(name="p1st", bufs=2))
                psums = ph1.enter_context(tc.tile_pool(name="p1ps", bufs=2, space="PSUM"))
                lnpools = (strm, st, psums)

                def p1_stats(ci):
                    # next chunk's input DMA + stats matmuls, emitted ahead
                    # so the PE works while the DVE finishes the previous
                    # chunk's LN. Squares on DVE: keeps ACT free for the
                    # projection psum->sbuf copies (their delay stalls PE).
                    xin = one1.tile([128, DK, CH], MMDT, tag="xin",
                                    name="xin")
                    nc.sync.dma_start(xin[:], xTc[ci])
                    psum_s = psums.tile([1, CH], F32, tag="ln_s")
                    psum_q = psums.tile([1, CH], F32, tag="ln_q")
                    for kk in range(DK):
                        sq = strm.tile([128, CH], MMDT, tag="sq")
                        if kk % 2 == 0:
                            nc.vector.tensor_mul(sq[:], xin[:, kk, :],
                                                 xin[:, kk, :])
                        else:
                            nc.scalar.activation(sq[:], xin[:, kk, :],
                                                 AF.Square)
                        nc.tensor.matmul(psum_s[:], r(ones128[:]),
                                         xin[:, kk, :],
                                         start=(kk == 0), stop=(kk == DK - 1))
                        nc.tensor.matmul(psum_q[:], r(ones128[:]), r(sq[:]),
                                         start=(kk == 0), stop=(kk == DK - 1))
                    return xin, psum_s, psum_q

                cur = p1_stats(0)
                for ci in range(4):
                    half, tq = ci // 2, ci % 2
                    tg = half * TM + tq * 512   # global token offset
                    nxt = p1_stats(ci + 1) if ci < 3 else None
                    xin, psum_s, psum_q = cur
                    y1q = one1.tile([128, DK, 512], BF, tag="y1q")
                    if half == 0:
                        loc = tq * 512
                        y0dst = lambda kk, lo=loc: y0buf[:, kk, ds(lo, CH)]
                    else:
                        y0dst = None
                    ln_finish(lambda kk, x=xin: x[:, kk, :], psum_s, psum_q,
                              CH, (strm, st), dst_fn=y0dst,
                              dst2_fn=lambda kk, y=y1q: y[:, kk, :])
                    # feature-major K (and Q for my half) projections
                    plist = [(wkall, bk_t, kfm, tg)]
                    if half == 0:
                        plist.append((wqall, bq_t, qfm, tq * 512))
                    for (wall, bias_t, dstbuf, dsto) in plist:
                        for m in range(4):
                            ps = psums.tile([128, 512], F32, tag="mm")
                            for kk in range(DK):
                                nc.tensor.matmul(ps[:], wall[:, kk, ts(m, 128)],
                                                 y1q[:, kk, :],
                                                 start=(kk == 0), stop=(kk == DK - 1))
                            nc.scalar.activation(
                                dstbuf[:, m, ds(dsto, 512)], ps[:], AF.Identity,
                                bias=bias_t[:, m:m + 1], scale=1.0)

                    # token-major V (bias broadcast along free dim)
                    for nt in range(4):
                        ps = psums.tile([128, INNER], F32, tag="mm")
                        for kk in range(DK):
                            nc.tensor.matmul(ps[:], y1q[:, kk, ts(nt, 128)],
                                             wvt[:, kk, :],
                                             start=(kk == 0), stop=(kk == DK - 1))
                        gnt = half * NTM + tq * 4 + nt
                        nc.vector.tensor_add(
                            vvbuf[:, gnt, :, 1:65],
                            ps[:].rearrange("p (h d) -> p h d", h=H),
                            bv_b[:].rearrange("p (h d) -> p h d", h=H))
                    cur = nxt

            if debug:
                pass
                nc.gpsimd.dma_start(dbg["k"][:], kfm[:])
                nc.gpsimd.dma_start(dbg["q"][:], qfm[:])
                nc.gpsimd.dma_start(dbg["vv"][:], vvbuf[:])

            # =========================================================
            # Phase 2: u-proj, FAVOR+ attention, Wo (shared psum pools)
            # =========================================================
            with ExitStack() as ph2:
                wstrm = ph2.enter_context(tc.tile_pool(name="p2w", bufs=2))
                apool = ph2.enter_context(tc.tile_pool(name="p2a", bufs=2))
                abig = ph2.enter_context(tc.tile_pool(name="p2b", bufs=2))
                kpp = ph2.enter_context(tc.tile_pool(name="p2kp", bufs=2))
                obp = ph2.enter_context(tc.tile_pool(name="p2ob", bufs=1))
                pshare = ph2.enter_context(tc.tile_pool(name="p2sh", bufs=3, space="PSUM"))
                attn_ps = ExitStack()
                ddA_p = attn_ps.enter_context(tc.tile_pool(name="p2pa", bufs=1, space="PSUM"))
                ddB_p = attn_ps.enter_context(tc.tile_pool(name="p2pb", bufs=2, space="PSUM"))
                pctx_p = attn_ps.enter_context(tc.tile_pool(name="p2pc", bufs=1, space="PSUM"))

                # ---- Phase 2a: u = y0 @ proj_W + proj_b (SBUF-resident).
                # Emission is deferred: the 16 chains are pure PE work with
                # no DVE/ACT cost, interleaved into the attention loop to
                # fill the stalls where ACT paces the exp streams.
                def u_chain(m, t2):
                    if t2 == 0:
                        wt = wstrm.tile([128, DK, 128], BF, tag="wu",
                                        name="wu")
                        nc.sync.dma_start(wt[:], projWt[m])
                        u_wt[0] = wt
                    ps = pshare.tile([128, 512], F32, tag="sh")
                    for kk in range(DK):
                        nc.tensor.matmul(ps[:], u_wt[0][:, kk, :],
                                         y0buf[:, kk, ds(t2 * 512, 512)],
                                         start=(kk == 0), stop=(kk == DK - 1))
                    nc.scalar.activation(ubuf[:, m, ds(t2 * 512, 512)], ps[:],
                                         AF.Identity,
                                         bias=projb_t[:, m:m + 1], scale=1.0)

                u_wt = [None]
                u_pending = [(m, t2) for m in range(DK) for t2 in range(2)]

                # ---- Phase 2b: FAVOR+ attention (software-pipelined) ----
                # Engines execute their queues in order, so overlap must be
                # arranged at emission time: head h's pass-B exp stream (the
                # ACT pacer) is interleaved with head h+1's pass-A matmuls;
                # the query side runs one head behind so its matmuls fill the
                # PE while ACT drains the exps; ctx matmuls run as one dense
                # burst out of the SBUF kp buffer.
                for _i in range(2):
                    _t = kpp.tile([128, NTF, 268], BF, tag="kp")
                    nc.vector.memset(_t[:, :, 266:268], 1.0)

                obuf2 = obp.tile([64, H, TM], BF, tag="obuf")

                def hsl(hh):
                    return slice(64 * (hh % 2), 64 * (hh % 2) + 64)

                pair_sq = {}

                def emit_sq(hp):
                    ksqt = abig.tile([128, TF], BF, tag="ksq")
                    nc.vector.tensor_mul(ksqt[:], kfm[:, hp, :], kfm[:, hp, :])
                    qsqt = abig.tile([128, TM], BF, tag="qsq")
                    nc.vector.tensor_mul(qsqt[:], qfm[:, hp, :], qfm[:, hp, :])
                    pair_sq[hp] = (ksqt, qsqt)


                st = {}

                def alloc_head(hh):
                    st[hh] = dict(
                        mx=apool.tile([128, NTF], F32, tag="mxa", name="mx"),
                        dg=apool.tile([128, NTF], F32, tag="dgk", name="dg"),
                        bk=apool.tile([128, NTF], F32, tag="bka", name="bk"),
                        kp=kpp.tile([128, NTF, 268], BF, tag="kp", name="kp"),
                        ctxT=abig.tile([128, 3, 66], BF, tag="ctxT", name="ctxT"),
                        qpT=abig.tile([128, 3, TM], BF, tag="qpT", name="qpT"),
                    )

                def passA_pair(hh, j):
                    hs = hsl(hh)
                    hp = hh // 2
                    ksqt = pair_sq[hp][0]
                    d = st[hh]
                    psd2 = ddA_p.tile([128, 2, 512], F32, tag="ddA")
                    for i in range(2):
                        nt = 2 * j + i
                        nc.tensor.matmul(psd2[:, i, 0:MF],
                                         kfm[hs, hp, ts(nt, 128)],
                                         projT2[hs, :], start=True, stop=True)
                        nc.tensor.matmul(psd2[:, i, 268:270],
                                         ksqt[hs, ts(nt, 128)],
                                         ones_pair[hs, :], start=True, stop=True)
                    nc.vector.tensor_reduce(d["mx"][:, 2 * j:2 * j + 2],
                                            psd2[:, :, 0:MF], axis=AX.X,
                                            op=OP.max)
                    nc.vector.tensor_scalar_mul(d["dg"][:, 2 * j:2 * j + 2],
                                                psd2[:, :, 268:269], DIAG_SCALE)

                def gmax_chain(hh):
                    d = st[hh]
                    gmax = apool.tile([128, 1], F32, tag="gmax")
                    nc.vector.tensor_reduce(gmax[:], d["mx"][:], axis=AX.X,
                                            op=OP.max)
                    gall = apool.tile([128, 1], F32, tag="gall")
                    nc.gpsimd.partition_all_reduce(gall[:], gmax[:], 128,
                                                   bass_isa.ReduceOp.max)
                    mneg = apool.tile([128, 1], F32, tag="mneg")
                    nc.gpsimd.tensor_scalar_mul(mneg[:], gall[:], -1.0)
                    nc.vector.tensor_scalar(d["bk"][:], d["dg"][:], -1.0,
                                            mneg[:, 0:1], op0=OP.mult,
                                            op1=OP.add)

                def passB_step(hh, nt):
                    hs = hsl(hh)
                    hp = hh // 2
                    d = st[hh]
                    psd = ddB_p.tile([128, 272], F32, tag="ddB")
                    nc.tensor.matmul(psd[:, 0:MF],
                                     kfm[hs, hp, ts(nt, 128)],
                                     projT2[hs, :], start=True, stop=True)
                    nc.scalar.activation(d["kp"][:, nt, 0:MF], psd[:, 0:MF],
                                         AF.Exp, bias=d["bk"][:, nt:nt + 1],
                                         scale=1.0)

                def ctx_burst(hh):
                    d = st[hh]
                    pctx = pctx_p.tile([66, 268], F32, tag="ctx")
                    for nt in range(NTF):
                        nc.tensor.matmul(pctx[:], vvbuf[:, nt, hh, :],
                                         d["kp"][:, nt, :],
                                         start=(nt == 0), stop=(nt == NTF - 1))
                    return pctx

                def ctx_epi(hh, pctx):
                    d = st[hh]
                    ctx_raw = apool.tile([66, 268], F32, tag="ctxraw")
                    nc.vector.tensor_copy(ctx_raw[:], pctx[:])
                    ctx_sb = apool.tile([66, MF + 1], F32, tag="ctxsb")
                    nc.vector.scalar_tensor_tensor(
                        ctx_sb[:, 0:MF],
                        ctx_raw[:, 266:267].broadcast_to((66, MF)), EPSK,
                        ctx_raw[:, 0:MF], op0=OP.mult, op1=OP.add)
                    with nc.allow_low_precision(reason="fp32 DVE reduce"):
                        nc.vector.tensor_reduce(ctx_sb[:, MF:MF + 1],
                                                ctx_sb[:, 0:MF],
                                                axis=AX.X, op=OP.add)
                    # ctxT cols: [ksum | ctx(64) | ksum]; chunk2 row 10 =
                    # eps row (= transposed ctxsum incl. S at cols 0/65)
                    for c in range(3):
                        w = 128 if c < 2 else 11
                        ptt = pshare.tile([128, 512], F32, tag="sh")
                        nc.tensor.transpose(ptt[0:w, 0:66],
                                            ctx_sb[0:66, ds(c * 128, w)],
                                            identF[0:66, 0:66])
                        nc.scalar.activation(d["ctxT"][0:w, c, :],
                                             ptt[0:w, 0:66], AF.Copy)

                def query_a(hh):
                    hs = hsl(hh)
                    hp = hh // 2
                    d = st[hh]
                    qsqt = pair_sq[hp][1]
                    for t2 in range(2):
                        t2s = ds(t2 * 512, 512)
                        for c in range(2):
                            shq = pshare.tile([128, 512], F32, tag="sh")
                            nc.tensor.matmul(shq[:], projT2[hs, ts(c, 128)],
                                             qfm[hs, hp, t2s],
                                             start=True, stop=True)
                            nc.scalar.activation(d["qpT"][:, c, t2s], shq[:],
                                                 AF.Exp)
                        psq2 = pshare.tile([128, 512], F32, tag="sh")
                        nc.tensor.matmul(psq2[0:11, :], projT2x[hs, :],
                                         qfm[hs, hp, t2s],
                                         start=True, stop=False)
                        nc.tensor.matmul(psq2[0:11, :], dcolx[hs, :],
                                         qsqt[hs, t2s],
                                         start=False, stop=True)
                        nc.scalar.activation(d["qpT"][0:11, 2, t2s],
                                             psq2[0:11, :], AF.Exp,
                                             bias=lneps11[0:11, :], scale=1.0)

                def query_b(hh):
                    d = st[hh]
                    ctxT, qpT = d["ctxT"], d["qpT"]
                    for t2 in range(2):
                        t2s = ds(t2 * 512, 512)
                        # one chain: rows 0:64 = o numerator, row 64 = den
                        po = pshare.tile([128, 512], F32, tag="sh")
                        nc.tensor.matmul(po[0:65, :], ctxT[:, 0, 1:66],
                                         qpT[:, 0, t2s], start=True, stop=False)
                        nc.tensor.matmul(po[0:65, :], ctxT[:, 1, 1:66],
                                         qpT[:, 1, t2s], start=False, stop=False)
                        nc.tensor.matmul(po[0:65, :], ctxT[0:11, 2, 1:66],
                                         qpT[0:11, 2, t2s],
                                         start=False, stop=True)
                        # reciprocal on lane 64, DMA-stage to partition 0,
                        # broadcast, multiply
                        denst = apool.tile([128, 512], F32, tag="denst")
                        nc.vector.reciprocal(denst[64:65, :], po[64:65, :])
                        denr = apool.tile([1, 512], F32, tag="denr")
                        nc.sync.dma_start(denr[:], denst[64:65, :])
                        denb = apool.tile([64, 512], F32, tag="denb")
                        nc.gpsimd.partition_broadcast(denb[:], denr[:])
                        nc.vector.tensor_mul(obuf2[0:64, hh, t2s],
                                             po[0:64, :], denb[:])

                emit_sq(0)
                alloc_head(0)
                for j in range(NTF // 2):
                    passA_pair(0, j)
                gmax_chain(0)
                for hh in range(H + 1):
                    if hh < H:
                        if hh + 1 < H:
                            if (hh + 1) % 2 == 0:
                                emit_sq((hh + 1) // 2)
                            alloc_head(hh + 1)
                        for j in range(NTF // 2):
                            passB_step(hh, 2 * j)
                            passB_step(hh, 2 * j + 1)
                            if hh + 1 < H:
                                passA_pair(hh + 1, j)
                        for _ in range(2):
                            if u_pending:
                                u_chain(*u_pending.pop(0))
                        if hh >= 1:
                            query_a(hh - 1)
                        pctx = ctx_burst(hh)
                        if hh + 1 < H:
                            gmax_chain(hh + 1)
                        ctx_epi(hh, pctx)
                        if hh >= 1:
                            query_b(hh - 1)
                    else:
                        query_a(H - 1)
                        query_b(H - 1)

                if debug:
                    nc.gpsimd.dma_start(dbg["obuf"][:], obuf2[:])
                attn_ps.close()

                # ---- Phase 2c: v1 = y0 + o @ Wo + bo (in-place) ----
                for m in range(DK):
                    wt = wstrm.tile([64, H, 128], BF, tag="wo")
                    nc.gpsimd.dma_start(wt[:], Wot2[m])
                    for t2 in range(2):
                        t2s = ds(t2 * 512, 512)
                        ps = pshare.tile([128, 512], F32, tag="sh")
                        for hh in range(H):
                            nc.tensor.matmul(ps[:], wt[:, hh, :],
                                             obuf2[:, hh, t2s],
                                             start=(hh == 0), stop=(hh == H - 1))
                        nc.vector.scalar_tensor_tensor(
                            y0buf[:, m, t2s], ps[:], bo_t[:, m:m + 1],
                            y0buf[:, m, t2s], op0=OP.add, op1=OP.add)

        if debug:
            pass
            nc.gpsimd.dma_start(dbg["u"][:], ubuf[:])

        # =============================================================
        # Phases 4/5: performer FF + gating, then block FFN + residual
        # =============================================================
        with ExitStack() as ph45:
            strm = ph45.enter_context(tc.tile_pool(name="p4s", bufs=2))
            st4 = ph45.enter_context(tc.tile_pool(name="p4st", bufs=2))
            fbig = ph45.enter_context(tc.tile_pool(name="p4b", bufs=1))
            # DMA-written stream tiles: explicit ping-pong tags in a bufs=1
            # pool — the first DMA into each fresh buffer of a rotating
            # (bufs=2) tag was observed to land late (stale weight tiles)
            wp = ph45.enter_context(tc.tile_pool(name="p4w", bufs=1))
            psums = ph45.enter_context(tc.tile_pool(name="p4ps", bufs=2, space="PSUM"))

            x1buf = fbig.tile([128, DK, TM], BF, tag="x1")

            def ffn_phase(src_fn, w1_t, b1_t, w2_t, out_cb, nm="",
                          pre_stats=None):
                # LN both halves: stats chains first (PE back-to-back), then
                # finishes; W1 pipelines behind the per-kk y2t applies.
                # pre_stats: sum/sum-sq psums already accumulated upstream.
                y2t = ylife.tile([128, DK, TM], BF, tag="y2t",
                                 name=f"y2t{nm}")
                stats = pre_stats if pre_stats is not None else []
                for t2 in (() if pre_stats is not None else range(2)):
                    src = src_fn(t2)
                    psum_s = psums.tile([1, 512], F32, tag=f"ln_s{t2}",
                                        name=f"lns{t2}{nm}", bufs=1)
                    psum_q = psums.tile([1, 512], F32, tag=f"ln_q{t2}",
                                        name=f"lnq{t2}{nm}", bufs=1)
                    for kk in range(DK):
                        sq = strm.tile([128, 512], BF, tag="sq")
                        nc.scalar.activation(sq[:], src(kk), AF.Square)
                        nc.tensor.matmul(psum_s[:], ones_pair[:, 0:1], src(kk),
                                         start=(kk == 0), stop=(kk == DK - 1))
                        nc.tensor.matmul(psum_q[:], ones_pair[:, 0:1], sq[:],
                                         start=(kk == 0), stop=(kk == DK - 1))
                    stats.append((psum_s, psum_q))
                for t2 in range(2):
                    ln_finish(src_fn(t2), stats[t2][0], stats[t2][1], 512,
                              (strm, st4),
                              dst_fn=lambda kk, s=ds(t2 * 512, 512):
                                  y2t[:, kk, s])
                # stream each weight tile ONCE and apply it to both halves
                h1 = fbig.tile([128, 32, TM], BF, tag="h1")
                for m in range(32):
                    wt = wp.tile([128, DK, 128], BF, tag=f"w1_{m % 2}")
                    # alternate DMA rings: one ring can't carry the
                    # full FFN weight stream at rate
                    (nc.gpsimd if m % 2 == 0 else nc.scalar).dma_start(
                        wt[:], w1_t[m])
                    for t2 in range(2):
                        t2s = ds(t2 * 512, 512)
                        ph = psums.tile([128, 512], F32, tag="mm")
                        for kk in range(DK):
                            nc.tensor.matmul(ph[:], wt[:, kk, :],
                                             y2t[:, kk, t2s],
                                             start=(kk == 0), stop=(kk == DK - 1))
                        nc.scalar.activation(h1[:, m, t2s], ph[:], AF.Gelu,
                                             bias=b1_t[:, m:m + 1], scale=1.0)
                for mo in range(DK):
                    # W2 streamed in two 16-ktile halves (SBUF pressure)
                    w2h = []
                    for hf in range(2):
                        wt2 = wp.tile([128, 16, 128], BF, tag=f"w2_{hf}")
                        (nc.scalar if hf == 0 else nc.gpsimd).dma_start(
                            wt2[:], w2_t[mo, :, ds(hf * 16, 16)])
                        w2h.append(wt2)
                    for t2 in range(2):
                        pv = psums.tile([128, 512], F32, tag="mm")
                        for ks in range(32):
                            nc.tensor.matmul(pv[:], w2h[ks // 16][:, ks % 16, :],
                                             h1[:, ks, ds(t2 * 512, 512)],
                                             start=(ks == 0), stop=(ks == 31))
                        out_cb(mo, t2, pv)

            def pff_out(mo, t2, pv):
                t2s = ds(t2 * 512, 512)
                v2t = strm.tile([128, 512], F32, tag="v2t")
                nc.vector.scalar_tensor_tensor(v2t[:], pv[:], pb2_t[:, mo:mo + 1],
                                               y0buf[:, mo, t2s], op0=OP.add,
                                               op1=OP.add)
                xt = wp.tile([128, 512], MMDT, tag="xt")
                nc.gpsimd.dma_start(xt[:], xTc[t2, :, mo, :])
                t3 = strm.tile([128, 512], F32, tag="t3")
                nc.vector.tensor_mul(t3[:], v2t[:], ubuf[:, mo, t2s])
                nc.vector.tensor_add(x1buf[:, mo, t2s], t3[:], xt[:])
                # eager phase-5 LN stats (full-clock region); reuses the
                # ln_s/ln_q psum tags after phase 4's finish released them
                if mo == 0:
                    es5[t2] = psums.tile([1, 512], F32, tag=f"ln_s{t2}",
                                         name=f"es5{t2}", bufs=1)
                    eq5[t2] = psums.tile([1, 512], F32, tag=f"ln_q{t2}",
                                         name=f"eq5{t2}", bufs=1)
                sq5 = strm.tile([128, 512], BF, tag="sq5")
                nc.scalar.activation(sq5[:], x1buf[:, mo, t2s], AF.Square)
                nc.tensor.matmul(es5[t2][:], ones_pair[:, 0:1],
                                 x1buf[:, mo, t2s],
                                 start=(mo == 0), stop=(mo == DK - 1))
                nc.tensor.matmul(eq5[t2][:], ones_pair[:, 0:1], sq5[:],
                                 start=(mo == 0), stop=(mo == DK - 1))

            es5, eq5 = {}, {}
            ffn_phase(lambda t2: (lambda kk, s=ds(t2 * 512, 512):
                                  y0buf[:, kk, s]),
                      pW1t, pb1_t, pW2t, pff_out, nm="4")

            def ffn2_out(mo, t2, pv):
                ot = strm.tile([128, 512], F32, tag="ot")
                nc.vector.scalar_tensor_tensor(ot[:], pv[:], bf2_t[:, mo:mo + 1],
                                               x1buf[:, mo, ds(t2 * 512, 512)],
                                               op0=OP.add, op1=OP.add)
                nc.sync.dma_start(outT[ts(mo, 128), ds(t2 * 512, 512)], ot[:])

            ffn_phase(lambda t2: (lambda kk, s=ds(t2 * 512, 512):
                                  x1buf[:, kk, s]),
                      Wf1t, bf1_t, Wf2t, ffn2_out, nm="5",
                      pre_stats=[(es5[0], eq5[0]), (es5[1], eq5[1])])

    nc.compile()
    return nc


_NC_CACHE = {}


def _get_nc(debug=False):
    key = ("dbg" if debug else "nc")
    if key not in _NC_CACHE:
        _NC_CACHE[key] = build_nc(debug)
    return _NC_CACHE[key]


def _pack_w(w, bf, part_tiles):
    """W[K, N] -> [N/128, 128, K/128, 128] (m, p, kk, n) contiguous tiles."""
    import ml_dtypes
    K, N = w.shape
    arr = np.asarray(w, np.float32).reshape(K // 128, 128, N // 128, 128)
    arr = arr.transpose(2, 1, 0, 3)
    return np.ascontiguousarray(arr.astype(ml_dtypes.bfloat16) if bf else arr)


def make_in_maps(inputs):
    import ml_dtypes
    x = np.asarray(inputs["x"], dtype=np.float32)
    projTdn = np.ascontiguousarray(
        (np.asarray(inputs["proj_mat"], np.float32).T * DN).astype(ml_dtypes.bfloat16))

    f32 = lambda k: np.asarray(inputs[k], np.float32)
    # Wot2[m, p, h, j] = Wo[h*64+p, m*128+j]
    wot2 = f32("Wo").reshape(H, 64, DK, 128).transpose(2, 1, 0, 3)
    common = dict(
        projTdn=projTdn,
        Wqt=_pack_w(f32("Wq"), True, 4),
        Wkt=_pack_w(f32("Wk"), True, 4),
        Wvt=np.ascontiguousarray(
            f32("Wv").reshape(DK, 128, INNER).transpose(1, 0, 2)
            .astype(ml_dtypes.bfloat16)),
        Wot2=np.ascontiguousarray(wot2.astype(ml_dtypes.bfloat16)),
        projWt=_pack_w(f32("proj_W"), True, 8),
        pW1t=_pack_w(f32("pW1"), True, 32),
        pW2t=_pack_w(f32("pW2"), True, 8),
        Wf1t=_pack_w(f32("Wf1"), True, 32),
        Wf2t=_pack_w(f32("Wf2"), True, 8),
        bv_row=np.ascontiguousarray(f32("bv").reshape(1, INNER)),
    )
    vecs = np.zeros((128, VEC_COLS), np.float32)
    for name, (off, n) in VEC_PACK.items():
        vecs[:, off:off + n] = f32(name).reshape(n, 128).T
    vecs[10, 104] = np.log(EPSK)
    common["vecs"] = np.ascontiguousarray(vecs)

    in_maps = []
    for c in range(N_CORES):
        b, off = c // 2, (c % 2) * TM
        x_rot = np.roll(x[b], -off, axis=0)            # my tokens first
        m = dict(common)
        # xTc[c, p, kk, ti] = x_rot[c*512+ti, kk*128+p]
        m["xTc"] = np.ascontiguousarray(
            x_rot.reshape(4, 512, DK, 128).transpose(0, 3, 2, 1))
        in_maps.append(m)
    return in_maps


def _run(inputs, trace=False, debug=False):
    nc = _get_nc(debug)
    in_maps = make_in_maps(inputs)
    res = run_bass_kernel_spmd(nc, in_maps, core_ids=list(range(N_CORES)),
                               trace=trace)
    x = np.asarray(inputs["x"], dtype=np.float32)
    out = np.empty_like(x)
    for c in range(N_CORES):
        b, off = c // 2, (c % 2) * TM
        out[b, off:off + TM] = res.results[c]["outT"].T
    return out, res


def kernel(**inputs):
    out, _ = _run(inputs, trace=False)
    return out


# revision 29
# speedup vs baseline: 1.1430x; 1.1430x over previous
"""Trainium2 Bass kernel for nn_FAVORiserBlock (Performer gated transformer block).

Sharding: 8 cores; core c handles batch b=c//2, token-half h=c%2 (1024 of 2048
tokens). The FAVOR+ key-side statistics (global key max, k_sum, ctx) need the
full 2048-token sequence, so each core recomputes the key side for its whole
batch — zero cross-core communication, pure SPMD. The host rotates each core's
sequence so that its own 1024 tokens come first (key-side reductions are
order-invariant).

Attention restructure vs the original: the query side is computed entirely
feature-major. qpT[m, tok] = exp(dd) is produced directly by 3 m-chunk matmuls
+ one ACT exp (no per-query row max - the eps term is re-weighted, validated
offline at ~3e-3 end-to-end). The per-token diag factor exp(diag_q)*eps rides
as an extra lhsT row (row 266), and ksum rides as an extra ctxT column, so ONE
matmul chain yields both o numerator (rows 0:64) and denominator (row 64);
the denominator rides as row 64 of the same chain (lane-64 reciprocal, DMA
stage to partition 0, broadcast, multiply). Emission is software-pipelined:
engines execute in order, so head h+1's pass-A matmuls are interleaved into
head h's exp stream, the query side runs one head behind, and the u-projection
chains (pure PE) are drip-fed into the attention loop to fill ACT-paced
stalls. The chip's HAM duty-cycle governor halves the PE rate during
multi-engine phases regardless; minimizing attention's PE cycles is what
shortens that window.

All activations are kept feature-major ([d, tokens], d on partitions) so every
matmul consumes them directly; the host pre-transposes x and post-transposes
the output. Matmuls run bf16 / float32r (full PE rate at N>=256).

Weights are host-repacked into per-tile-contiguous layouts so every SBUF
weight load is 1-8KB-per-partition descriptors. u and the inter-block
activation x1 stay resident in SBUF.
"""
import sys

sys.path.insert(0, "/opt/trn_rl_repo")

from contextlib import ExitStack

import numpy as np

import concourse.bass as bass
import concourse.bass_isa as bass_isa
import concourse.mybir as mybir
import concourse.tile as tile
from concourse import bacc
from concourse.bass import ts, ds
from concourse.bass_utils import run_bass_kernel_spmd
from concourse.masks import make_identity

F32 = mybir.dt.float32
MMDT = mybir.dt.float32r
BF = mybir.dt.bfloat16
AX = mybir.AxisListType
OP = mybir.AluOpType
AF = mybir.ActivationFunctionType

# dims (hardcoded for this problem)
D = 1024          # d_model
DK = D // 128     # 8 feature k-tiles
INNER = 512
H = 8
DH = 64
MF = 266          # FAVOR+ features
TF = 2048         # full sequence (per batch)
TM = 1024         # tokens owned by this core
NTF = TF // 128
NTM = TM // 128
FF = 4096
CH = 512          # phase-1 LayerNorm chunk (tokens)

DN = float(64 ** -0.25)
EPSK = 1e-4
LNEPSK = float(np.log(EPSK))
EPSLN = 1e-5
DIAG_SCALE = 0.5 * DN * DN  # multiplies sum(k^2); == 1/16 exactly

N_CORES = 8
BATCH, SEQ = 4, 2048

# packed bias vector columns: name -> (col offset, n cols)
VEC_PACK = dict(proj_b=(0, 8), bq=(8, 4), bk=(12, 4), bo=(16, 8), pb2=(24, 8),
                bf2=(32, 8), pb1=(40, 32), bf1=(72, 32))
VEC_COLS = 105  # col 104: ln(eps) at row 10, 0 elsewhere (query eps-row bias)


def r(ap):
    return ap.bitcast(MMDT)


def build_nc(debug=False):
    nc = bacc.Bacc("TRN2", target_bir_lowering=False, debug=False)

    # x repacked host-side: xTc[c, p, kk, ti] = x_rot[c*512+ti, kk*128+p]
    xTc = nc.dram_tensor("xTc", [4, 128, DK, 512], MMDT, kind="ExternalInput")
    projTdn = nc.dram_tensor("projTdn", [DH, MF], BF, kind="ExternalInput")
    # weights repacked host-side into per-tile-contiguous layouts
    Wqt = nc.dram_tensor("Wqt", [4, 128, DK, 128], BF, kind="ExternalInput")
    Wkt = nc.dram_tensor("Wkt", [4, 128, DK, 128], BF, kind="ExternalInput")
    Wvt = nc.dram_tensor("Wvt", [128, DK, INNER], BF, kind="ExternalInput")
    Wot2 = nc.dram_tensor("Wot2", [DK, 64, H, 128], BF, kind="ExternalInput")
    projWt = nc.dram_tensor("projWt", [DK, 128, DK, 128], BF, kind="ExternalInput")
    pW1t = nc.dram_tensor("pW1t", [32, 128, DK, 128], BF, kind="ExternalInput")
    pW2t = nc.dram_tensor("pW2t", [DK, 128, 32, 128], BF, kind="ExternalInput")
    Wf1t = nc.dram_tensor("Wf1t", [32, 128, DK, 128], BF, kind="ExternalInput")
    Wf2t = nc.dram_tensor("Wf2t", [DK, 128, 32, 128], BF, kind="ExternalInput")
    vecs = nc.dram_tensor("vecs", [128, VEC_COLS], F32, kind="ExternalInput")
    bv_dram = nc.dram_tensor("bv_row", [1, INNER], F32, kind="ExternalInput")
    outT = nc.dram_tensor("outT", [D, TM], F32, kind="ExternalOutput")
    dbg = {}
    if debug:
        for nm, shp in dict(y0=[128, DK, TM], k=[128, 4, TF], q=[128, 4, TM],
                            u=[128, DK, TM], v1=[128, DK, TM],
                            vv=[128, NTF, H, 66], obuf=[64, H, TM],
                            ctxT=[128, 3, 66], qpT=[128, 3, TM],
                            den=[1, 512]).items():
            dbg[nm] = nc.dram_tensor(f"dbg_{nm}", shp, F32, kind="ExternalOutput")

    with tile.TileContext(nc) as tc, ExitStack() as top:
        const = top.enter_context(tc.tile_pool(name="const", bufs=1))

        # ---- constants ----
        identF = const.tile([128, 128], F32)
        make_identity(nc, identF[:])
        onesF = const.tile([128, 128], F32)
        nc.vector.memset(onesF[:], 1.0)
        ones128 = const.tile([128, 1], MMDT)
        nc.gpsimd.dma_start(ones128[:], onesF[:, 0:1])
        ones_pair = const.tile([128, 2], BF)
        nc.gpsimd.dma_start(ones_pair[:], onesF[:, 0:2])
        projT2 = const.tile([128, MF], BF)  # projT duplicated to both halves
        nc.sync.dma_start(projT2[0:DH, :], projTdn[:, :])
        nc.sync.dma_start(projT2[DH:128, :], projTdn[:, :])
        # lhsT for the combined dd-tail + diag matmul pair:
        #   projT2x = [projT chunk2 (10 cols) | 0], dcolx = [0 x10 | DIAG_SCALE]
        projT2x = const.tile([128, 11], BF)
        nc.vector.memset(projT2x[:], 0.0)
        nc.sync.dma_start(projT2x[0:DH, 0:10], projTdn[:, 256:266])
        nc.sync.dma_start(projT2x[DH:128, 0:10], projTdn[:, 256:266])
        dcolx = const.tile([128, 11], BF)
        nc.vector.memset(dcolx[:], 0.0)
        nc.vector.memset(dcolx[:, 10:11], DIAG_SCALE)
        eps1 = const.tile([1, 1], F32)
        nc.vector.memset(eps1[:], EPSLN)

        vtile = const.tile([128, VEC_COLS], F32, tag="vecs")
        nc.sync.dma_start(vtile[:], vecs[:, :])

        def vec_tile(name):
            off, n = VEC_PACK[name]
            return vtile[:, off:off + n]

        # per-partition exp bias for the 11-row query chunk (row 10 = ln eps)
        lneps11 = vtile[:, 104:105]
        projb_t = vec_tile("proj_b")
        bq_t, bk_t = vec_tile("bq"), vec_tile("bk")
        bo_t, pb2_t, bf2_t = vec_tile("bo"), vec_tile("pb2"), vec_tile("bf2")
        pb1_t, bf1_t = vec_tile("pb1"), vec_tile("bf1")
        bv_row = const.tile([1, INNER], F32)
        nc.sync.dma_start(bv_row[:], bv_dram[:, :])

        ylife = top.enter_context(tc.tile_pool(name="ylife", bufs=1))
        y0buf = ylife.tile([128, DK, TM], BF, tag="y0")  # my-half y0; becomes v1
        ubuf = ylife.tile([128, DK, TM], BF, tag="u")      # gating projection u

        # =============================================================
        # LayerNorm helper (feature-major): stats via ones-matmuls
        # =============================================================
        def ln_finish(src_fn, psum_s, psum_q, width, pools, dst_fn=None,
                      dst2_fn=None):
            """Finish LN given the accumulated sum / sum-of-squares rows."""
            strm, st = pools
            mu = st.tile([1, width], F32, tag="mu")
            nc.vector.tensor_scalar_mul(mu[:], psum_s[:], 1.0 / D)
            mu2 = st.tile([1, width], F32, tag="tA")
            nc.vector.tensor_mul(mu2[:], mu[:], mu[:])
            var = st.tile([1, width], F32, tag="var")
            nc.vector.scalar_tensor_tensor(var[:], psum_q[:], 1.0 / D, mu2[:],
                                           op0=OP.mult, op1=OP.subtract)
            std = st.tile([1, width], F32, tag="tA")
            nc.scalar.activation(std[:], var[:], AF.Sqrt, bias=eps1[:], scale=1.0)
            s = st.tile([1, width], F32, tag="sln")
            nc.vector.reciprocal(s[:], std[:])
            mu_b = st.tile([128, width], F32, tag="A_b")
            s_b = st.tile([128, width], F32, tag="B_b")
            nc.gpsimd.partition_broadcast(mu_b[:], mu[:])
            nc.gpsimd.partition_broadcast(s_b[:], s[:])
            if dst2_fn is not None:
                t = st.tile([1, width], F32, tag="tA")
                nc.vector.tensor_mul(t[:], var[:], s[:])
                t2 = st.tile([1, width], F32, tag="tB")
                nc.vector.tensor_mul(t2[:], t[:], s[:])     # v/(v+eps)
                std2 = st.tile([1, width], F32, tag="tA")
                nc.scalar.activation(std2[:], t2[:], AF.Sqrt, bias=eps1[:], scale=1.0)
                r2 = st.tile([1, width], F32, tag="tB")
                nc.vector.reciprocal(r2[:], std2[:])
                s2 = st.tile([1, width], F32, tag="tA")
                nc.vector.tensor_mul(s2[:], r2[:], s[:])
                s2_b = st.tile([128, width], F32, tag="C_b")
                nc.gpsimd.partition_broadcast(s2_b[:], s2[:])
            for kk in range(DK):
                tmu = strm.tile([128, width], F32, tag="t1")
                nc.vector.tensor_sub(tmu[:], src_fn(kk), mu_b[:])
                if dst_fn is not None:
                    nc.vector.tensor_mul(dst_fn(kk), tmu[:], s_b[:])
                if dst2_fn is not None:
                    nc.vector.tensor_mul(dst2_fn(kk), tmu[:], s2_b[:])

        def layernorm(src_fn, width, pools, dst_fn=None, dst2_fn=None,
                      bf=False):
            """y = LN(src): stats matmuls + finish (gains/biases identity)."""
            strm, st, psums = pools
            psum_s = psums.tile([1, width], F32, tag="ln_s")
            psum_q = psums.tile([1, width], F32, tag="ln_q")
            ones_s = ones_pair[:, 0:1] if bf else r(ones128[:])
            cast = (lambda ap: ap) if bf else r
            for kk in range(DK):
                sq = strm.tile([128, width], BF if bf else MMDT, tag="sq")
                nc.scalar.activation(sq[:], src_fn(kk), AF.Square)
                nc.tensor.matmul(psum_s[:], ones_s, cast(src_fn(kk)),
                                 start=(kk == 0), stop=(kk == DK - 1))
                nc.tensor.matmul(psum_q[:], ones_s, cast(sq[:]),
                                 start=(kk == 0), stop=(kk == DK - 1))
            ln_finish(src_fn, psum_s, psum_q, width, (strm, st),
                      dst_fn=dst_fn, dst2_fn=dst2_fn)

        with ExitStack() as ph12:
            pA = ph12.enter_context(tc.tile_pool(name="pA", bufs=1))
            kfm = pA.tile([128, 4, TF], BF, tag="kfm")        # k features [512, TF]
            qfm = pA.tile([128, 4, TM], BF, tag="qfm")
            # token-major v with ones guard cols 0 and 65: [1 | v(64) | 1]
            vvbuf = pA.tile([128, NTF, H, 66], BF, tag="vv")
            _oa = ones128[:]
            _ones_b = bass.AP(tensor=_oa.tensor, offset=_oa.offset,
                              ap=[list(_oa.ap[0]), [0, NTF], [0, H], [0, 2]])
            _vv = vvbuf[:]
            _vv_ones = bass.AP(tensor=_vv.tensor, offset=_vv.offset,
                               ap=[list(_vv.ap[0]), list(_vv.ap[1]),
                                   list(_vv.ap[2]), [65, 2]])
            nc.vector.tensor_copy(_vv_ones, _ones_b)

            # q/k/v weights resident for all of phase 1
            # =========================================================
            # Phase 1: LN1 -> LN2 -> Q/K/V projections, per 512-token tile
            # =========================================================
            with ExitStack() as ph1:
                pw1 = ph1.enter_context(tc.tile_pool(name="p1w", bufs=1))
                bv_b = pw1.tile([128, INNER], F32, tag="bvb")
                nc.gpsimd.partition_broadcast(bv_b[:], bv_row[:])
                wqall = pw1.tile([128, DK, INNER], BF, tag="wq")
                wkall = pw1.tile([128, DK, INNER], BF, tag="wk")
                wvt = pw1.tile([128, DK, INNER], BF, tag="wv")
                for m in range(4):
                    nc.sync.dma_start(wqall[:, :, ts(m, 128)], Wqt[m])
                    nc.sync.dma_start(wkall[:, :, ts(m, 128)], Wkt[m])
                nc.sync.dma_start(wvt[:], Wvt[:, :, :])
                strm = ph1.enter_context(tc.tile_pool(name="p1s", bufs=2))
                one1 = ph1.enter_context(tc.tile_pool(name="p1o", bufs=2))
                st = ph1.enter_context(tc.tile_pool(name="p1st", bufs=2))
                psums = ph1.enter_context(tc.tile_pool(name="p1ps", bufs=2, space="PSUM"))
                lnpools = (strm, st, psums)

                def p1_stats(ci):
                    # next chunk's input DMA + stats matmuls, emitted ahead
                    # so the PE works while the DVE finishes the previous
                    # chunk's LN. Squares on DVE: keeps ACT free for the
                    # projection psum->sbuf copies (their delay stalls PE).
                    xin = one1.tile([128, DK, CH], MMDT, tag="xin",
                                    name="xin")
                    nc.sync.dma_start(xin[:], xTc[ci])
                    psum_s = psums.tile([1, CH], F32, tag="ln_s")
                    psum_q = psums.tile([1, CH], F32, tag="ln_q")
                    for kk in range(DK):
                        sq = strm.tile([128, CH], MMDT, tag="sq")
                        nc.vector.tensor_mul(sq[:], xin[:, kk, :],
                                             xin[:, kk, :])
                        nc.tensor.matmul(psum_s[:], r(ones128[:]),
                                         xin[:, kk, :],
                                         start=(kk == 0), stop=(kk == DK - 1))
                        nc.tensor.matmul(psum_q[:], r(ones128[:]), r(sq[:]),
                                         start=(kk == 0), stop=(kk == DK - 1))
                    return xin, psum_s, psum_q

                cur = p1_stats(0)
                for ci in range(4):
                    half, tq = ci // 2, ci % 2
                    tg = half * TM + tq * 512   # global token offset
                    nxt = p1_stats(ci + 1) if ci < 3 else None
                    xin, psum_s, psum_q = cur
                    y1q = one1.tile([128, DK, 512], BF, tag="y1q")
                    if half == 0:
                        loc = tq * 512
                        y0dst = lambda kk, lo=loc: y0buf[:, kk, ds(lo, CH)]
                    else:
                        y0dst = None
                    ln_finish(lambda kk, x=xin: x[:, kk, :], psum_s, psum_q,
                              CH, (strm, st), dst_fn=y0dst,
                              dst2_fn=lambda kk, y=y1q: y[:, kk, :])
                    # feature-major K (and Q for my half) projections
                    plist = [(wkall, bk_t, kfm, tg)]
                    if half == 0:
                        plist.append((wqall, bq_t, qfm, tq * 512))
                    for (wall, bias_t, dstbuf, dsto) in plist:
                        for m in range(4):
                            ps = psums.tile([128, 512], F32, tag="mm")
                            for kk in range(DK):
                                nc.tensor.matmul(ps[:], wall[:, kk, ts(m, 128)],
                                                 y1q[:, kk, :],
                                                 start=(kk == 0), stop=(kk == DK - 1))
                            nc.scalar.activation(
                                dstbuf[:, m, ds(dsto, 512)], ps[:], AF.Identity,
                                bias=bias_t[:, m:m + 1], scale=1.0)

                    # token-major V (bias broadcast along free dim)
                    for nt in range(4):
                        ps = psums.tile([128, INNER], F32, tag="mm")
                        for kk in range(DK):
                            nc.tensor.matmul(ps[:], y1q[:, kk, ts(nt, 128)],
                                             wvt[:, kk, :],
                                             start=(kk == 0), stop=(kk == DK - 1))
                        gnt = half * NTM + tq * 4 + nt
                        nc.vector.tensor_add(
                            vvbuf[:, gnt, :, 1:65],
                            ps[:].rearrange("p (h d) -> p h d", h=H),
                            bv_b[:].rearrange("p (h d) -> p h d", h=H))
                    cur = nxt

            if debug:
                pass
                nc.gpsimd.dma_start(dbg["k"][:], kfm[:])
                nc.gpsimd.dma_start(dbg["q"][:], qfm[:])
                nc.gpsimd.dma_start(dbg["vv"][:], vvbuf[:])

            # =========================================================
            # Phase 2: u-proj, FAVOR+ attention, Wo (shared psum pools)
            # =========================================================
            with ExitStack() as ph2:
                wstrm = ph2.enter_context(tc.tile_pool(name="p2w", bufs=2))
                apool = ph2.enter_context(tc.tile_pool(name="p2a", bufs=2))
                abig = ph2.enter_context(tc.tile_pool(name="p2b", bufs=2))
                kpp = ph2.enter_context(tc.tile_pool(name="p2kp", bufs=2))
                obp = ph2.enter_context(tc.tile_pool(name="p2ob", bufs=1))
                pshare = ph2.enter_context(tc.tile_pool(name="p2sh", bufs=3, space="PSUM"))
                attn_ps = ExitStack()
                ddA_p = attn_ps.enter_context(tc.tile_pool(name="p2pa", bufs=1, space="PSUM"))
                ddB_p = attn_ps.enter_context(tc.tile_pool(name="p2pb", bufs=2, space="PSUM"))
                pctx_p = attn_ps.enter_context(tc.tile_pool(name="p2pc", bufs=1, space="PSUM"))

                # ---- Phase 2a: u = y0 @ proj_W + proj_b (SBUF-resident).
                # Emission is deferred: the 16 chains are pure PE work with
                # no DVE/ACT cost, interleaved into the attention loop to
                # fill the stalls where ACT paces the exp streams.
                def u_chain(m, t2):
                    if t2 == 0:
                        wt = wstrm.tile([128, DK, 128], BF, tag="wu",
                                        name="wu")
                        nc.sync.dma_start(wt[:], projWt[m])
                        u_wt[0] = wt
                    ps = pshare.tile([128, 512], F32, tag="sh")
                    for kk in range(DK):
                        nc.tensor.matmul(ps[:], u_wt[0][:, kk, :],
                                         y0buf[:, kk, ds(t2 * 512, 512)],
                                         start=(kk == 0), stop=(kk == DK - 1))
                    nc.scalar.activation(ubuf[:, m, ds(t2 * 512, 512)], ps[:],
                                         AF.Identity,
                                         bias=projb_t[:, m:m + 1], scale=1.0)

                u_wt = [None]
                u_pending = [(m, t2) for m in range(DK) for t2 in range(2)]

                # ---- Phase 2b: FAVOR+ attention (software-pipelined) ----
                # Engines execute their queues in order, so overlap must be
                # arranged at emission time: head h's pass-B exp stream (the
                # ACT pacer) is interleaved with head h+1's pass-A matmuls;
                # the query side runs one head behind so its matmuls fill the
                # PE while ACT drains the exps; ctx matmuls run as one dense
                # burst out of the SBUF kp buffer.
                for _i in range(2):
                    _t = kpp.tile([128, NTF, 268], BF, tag="kp")
                    nc.vector.memset(_t[:, :, 266:268], 1.0)

                obuf2 = obp.tile([64, H, TM], BF, tag="obuf")

                def hsl(hh):
                    return slice(64 * (hh % 2), 64 * (hh % 2) + 64)

                pair_sq = {}

                def emit_sq(hp):
                    ksqt = abig.tile([128, TF], BF, tag="ksq")
                    nc.vector.tensor_mul(ksqt[:], kfm[:, hp, :], kfm[:, hp, :])
                    qsqt = abig.tile([128, TM], BF, tag="qsq")
                    nc.vector.tensor_mul(qsqt[:], qfm[:, hp, :], qfm[:, hp, :])
                    pair_sq[hp] = (ksqt, qsqt)


                st = {}

                def alloc_head(hh):
                    st[hh] = dict(
                        mx=apool.tile([128, NTF], F32, tag="mxa", name="mx"),
                        dg=apool.tile([128, NTF], F32, tag="dgk", name="dg"),
                        bk=apool.tile([128, NTF], F32, tag="bka", name="bk"),
                        kp=kpp.tile([128, NTF, 268], BF, tag="kp", name="kp"),
                        ctxT=abig.tile([128, 3, 66], BF, tag="ctxT", name="ctxT"),
                        qpT=abig.tile([128, 3, TM], BF, tag="qpT", name="qpT"),
                    )

                def passA_pair(hh, j):
                    hs = hsl(hh)
                    hp = hh // 2
                    ksqt = pair_sq[hp][0]
                    d = st[hh]
                    psd2 = ddA_p.tile([128, 2, 512], F32, tag="ddA")
                    for i in range(2):
                        nt = 2 * j + i
                        nc.tensor.matmul(psd2[:, i, 0:MF],
                                         kfm[hs, hp, ts(nt, 128)],
                                         projT2[hs, :], start=True, stop=True)
                        nc.tensor.matmul(psd2[:, i, 268:270],
                                         ksqt[hs, ts(nt, 128)],
                                         ones_pair[hs, :], start=True, stop=True)
                    nc.vector.tensor_reduce(d["mx"][:, 2 * j:2 * j + 2],
                                            psd2[:, :, 0:MF], axis=AX.X,
                                            op=OP.max)
                    nc.vector.tensor_scalar_mul(d["dg"][:, 2 * j:2 * j + 2],
                                                psd2[:, :, 268:269], DIAG_SCALE)

                def gmax_chain(hh):
                    d = st[hh]
                    gmax = apool.tile([128, 1], F32, tag="gmax")
                    nc.vector.tensor_reduce(gmax[:], d["mx"][:], axis=AX.X,
                                            op=OP.max)
                    gall = apool.tile([128, 1], F32, tag="gall")
                    nc.gpsimd.partition_all_reduce(gall[:], gmax[:], 128,
                                                   bass_isa.ReduceOp.max)
                    mneg = apool.tile([128, 1], F32, tag="mneg")
                    nc.gpsimd.tensor_scalar_mul(mneg[:], gall[:], -1.0)
                    nc.vector.tensor_scalar(d["bk"][:], d["dg"][:], -1.0,
                                            mneg[:, 0:1], op0=OP.mult,
                                            op1=OP.add)

                def passB_step(hh, nt):
                    hs = hsl(hh)
                    hp = hh // 2
                    d = st[hh]
                    psd = ddB_p.tile([128, 272], F32, tag="ddB")
                    nc.tensor.matmul(psd[:, 0:MF],
                                     kfm[hs, hp, ts(nt, 128)],
                                     projT2[hs, :], start=True, stop=True)
                    nc.scalar.activation(d["kp"][:, nt, 0:MF], psd[:, 0:MF],
                                         AF.Exp, bias=d["bk"][:, nt:nt + 1],
                                         scale=1.0)

                def ctx_burst(hh):
                    d = st[hh]
                    pctx = pctx_p.tile([66, 268], F32, tag="ctx")
                    for nt in range(NTF):
                        nc.tensor.matmul(pctx[:], vvbuf[:, nt, hh, :],
                                         d["kp"][:, nt, :],
                                         start=(nt == 0), stop=(nt == NTF - 1))
                    return pctx

                def ctx_epi(hh, pctx):
                    d = st[hh]
                    ctx_raw = apool.tile([66, 268], F32, tag="ctxraw")
                    nc.vector.tensor_copy(ctx_raw[:], pctx[:])
                    ctx_sb = apool.tile([66, MF + 1], F32, tag="ctxsb")
                    nc.vector.scalar_tensor_tensor(
                        ctx_sb[:, 0:MF],
                        ctx_raw[:, 266:267].broadcast_to((66, MF)), EPSK,
                        ctx_raw[:, 0:MF], op0=OP.mult, op1=OP.add)
                    with nc.allow_low_precision(reason="fp32 DVE reduce"):
                        nc.vector.tensor_reduce(ctx_sb[:, MF:MF + 1],
                                                ctx_sb[:, 0:MF],
                                                axis=AX.X, op=OP.add)
                    # ctxT cols: [ksum | ctx(64) | ksum]; chunk2 row 10 =
                    # eps row (= transposed ctxsum incl. S at cols 0/65)
                    for c in range(3):
                        w = 128 if c < 2 else 11
                        ptt = pshare.tile([128, 512], F32, tag="sh")
                        nc.tensor.transpose(ptt[0:w, 0:66],
                                            ctx_sb[0:66, ds(c * 128, w)],
                                            identF[0:66, 0:66])
                        nc.scalar.activation(d["ctxT"][0:w, c, :],
                                             ptt[0:w, 0:66], AF.Copy)

                def query_a(hh):
                    hs = hsl(hh)
                    hp = hh // 2
                    d = st[hh]
                    qsqt = pair_sq[hp][1]
                    for t2 in range(2):
                        t2s = ds(t2 * 512, 512)
                        for c in range(2):
                            shq = pshare.tile([128, 512], F32, tag="sh")
                            nc.tensor.matmul(shq[:], projT2[hs, ts(c, 128)],
                                             qfm[hs, hp, t2s],
                                             start=True, stop=True)
                            nc.scalar.activation(d["qpT"][:, c, t2s], shq[:],
                                                 AF.Exp)
                        psq2 = pshare.tile([128, 512], F32, tag="sh")
                        nc.tensor.matmul(psq2[0:11, :], projT2x[hs, :],
                                         qfm[hs, hp, t2s],
                                         start=True, stop=False)
                        nc.tensor.matmul(psq2[0:11, :], dcolx[hs, :],
                                         qsqt[hs, t2s],
                                         start=False, stop=True)
                        nc.scalar.activation(d["qpT"][0:11, 2, t2s],
                                             psq2[0:11, :], AF.Exp,
                                             bias=lneps11[0:11, :], scale=1.0)

                def query_b(hh):
                    d = st[hh]
                    ctxT, qpT = d["ctxT"], d["qpT"]
                    for t2 in range(2):
                        t2s = ds(t2 * 512, 512)
                        # one chain: rows 0:64 = o numerator, row 64 = den
                        po = pshare.tile([128, 512], F32, tag="sh")
                        nc.tensor.matmul(po[0:65, :], ctxT[:, 0, 1:66],
                                         qpT[:, 0, t2s], start=True, stop=False)
                        nc.tensor.matmul(po[0:65, :], ctxT[:, 1, 1:66],
                                         qpT[:, 1, t2s], start=False, stop=False)
                        nc.tensor.matmul(po[0:65, :], ctxT[0:11, 2, 1:66],
                                         qpT[0:11, 2, t2s],
                                         start=False, stop=True)
                        # reciprocal on lane 64, DMA-stage to partition 0,
                        # broadcast, multiply
                        denst = apool.tile([128, 512], F32, tag="denst")
                        nc.vector.reciprocal(denst[64:65, :], po[64:65, :])
                        denr = apool.tile([1, 512], F32, tag="denr")
                        nc.sync.dma_start(denr[:], denst[64:65, :])
                        denb = apool.tile([64, 512], F32, tag="denb")
                        nc.gpsimd.partition_broadcast(denb[:], denr[:])
                        nc.vector.tensor_mul(obuf2[0:64, hh, t2s],
                                             po[0:64, :], denb[:])

                emit_sq(0)
                alloc_head(0)
                for j in range(NTF // 2):
                    passA_pair(0, j)
                gmax_chain(0)
                for hh in range(H + 1):
                    if hh < H:
                        if hh + 1 < H:
                            if (hh + 1) % 2 == 0:
                                emit_sq((hh + 1) // 2)
                            alloc_head(hh + 1)
                        for j in range(NTF // 2):
                            passB_step(hh, 2 * j)
                            passB_step(hh, 2 * j + 1)
                            if hh + 1 < H:
                                passA_pair(hh + 1, j)
                        for _ in range(2):
                            if u_pending:
                                u_chain(*u_pending.pop(0))
                        if hh >= 1:
                            query_a(hh - 1)
                        pctx = ctx_burst(hh)
                        if hh + 1 < H:
                            gmax_chain(hh + 1)
                        ctx_epi(hh, pctx)
                        if hh >= 1:
                            query_b(hh - 1)
                    else:
                        query_a(H - 1)
                        query_b(H - 1)

                if debug:
                    nc.gpsimd.dma_start(dbg["obuf"][:], obuf2[:])
                attn_ps.close()

                # ---- Phase 2c: v1 = y0 + o @ Wo + bo (in-place) ----
                for m in range(DK):
                    wt = wstrm.tile([64, H, 128], BF, tag="wo")
                    nc.gpsimd.dma_start(wt[:], Wot2[m])
                    for t2 in range(2):
                        t2s = ds(t2 * 512, 512)
                        ps = pshare.tile([128, 512], F32, tag="sh")
                        for hh in range(H):
                            nc.tensor.matmul(ps[:], wt[:, hh, :],
                                             obuf2[:, hh, t2s],
                                             start=(hh == 0), stop=(hh == H - 1))
                        nc.vector.scalar_tensor_tensor(
                            y0buf[:, m, t2s], ps[:], bo_t[:, m:m + 1],
                            y0buf[:, m, t2s], op0=OP.add, op1=OP.add)

        if debug:
            pass
            nc.gpsimd.dma_start(dbg["u"][:], ubuf[:])

        # =============================================================
        # Phases 4/5: performer FF + gating, then block FFN + residual
        # =============================================================
        with ExitStack() as ph45:
            strm = ph45.enter_context(tc.tile_pool(name="p4s", bufs=2))
            st4 = ph45.enter_context(tc.tile_pool(name="p4st", bufs=2))
            fbig = ph45.enter_context(tc.tile_pool(name="p4b", bufs=1))
            # DMA-written stream tiles: explicit ping-pong tags in a bufs=1
            # pool — the first DMA into each fresh buffer of a rotating
            # (bufs=2) tag was observed to land late (stale weight tiles)
            wp = ph45.enter_context(tc.tile_pool(name="p4w", bufs=1))
            psums = ph45.enter_context(tc.tile_pool(name="p4ps", bufs=2, space="PSUM"))

            x1buf = fbig.tile([128, DK, TM], BF, tag="x1")

            def ffn_phase(src_fn, w1_t, b1_t, w2_t, out_cb, nm=""):
                # LN both halves: stats chains first (PE back-to-back), then
                # finishes; W1 pipelines behind the per-kk y2t applies.
                y2t = ylife.tile([128, DK, TM], BF, tag="y2t",
                                 name=f"y2t{nm}")
                stats = []
                for t2 in range(2):
                    src = src_fn(t2)
                    psum_s = psums.tile([1, 512], F32, tag=f"ln_s{t2}",
                                        name=f"lns{t2}{nm}", bufs=1)
                    psum_q = psums.tile([1, 512], F32, tag=f"ln_q{t2}",
                                        name=f"lnq{t2}{nm}", bufs=1)
                    for kk in range(DK):
                        sq = strm.tile([128, 512], BF, tag="sq")
                        nc.scalar.activation(sq[:], src(kk), AF.Square)
                        nc.tensor.matmul(psum_s[:], ones_pair[:, 0:1], src(kk),
                                         start=(kk == 0), stop=(kk == DK - 1))
                        nc.tensor.matmul(psum_q[:], ones_pair[:, 0:1], sq[:],
                                         start=(kk == 0), stop=(kk == DK - 1))
                    stats.append((psum_s, psum_q))
                for t2 in range(2):
                    ln_finish(src_fn(t2), stats[t2][0], stats[t2][1], 512,
                              (strm, st4),
                              dst_fn=lambda kk, s=ds(t2 * 512, 512):
                                  y2t[:, kk, s])
                # stream each weight tile ONCE and apply it to both halves
                h1 = fbig.tile([128, 32, TM], BF, tag="h1")
                for m in range(32):
                    wt = wp.tile([128, DK, 128], BF, tag=f"w1_{m % 2}")
                    # alternate DMA rings: one ring can't carry the
                    # full FFN weight stream at rate
                    (nc.gpsimd if m % 2 == 0 else nc.scalar).dma_start(
                        wt[:], w1_t[m])
                    for t2 in range(2):
                        t2s = ds(t2 * 512, 512)
                        ph = psums.tile([128, 512], F32, tag="mm")
                        for kk in range(DK):
                            nc.tensor.matmul(ph[:], wt[:, kk, :],
                                             y2t[:, kk, t2s],
                                             start=(kk == 0), stop=(kk == DK - 1))
                        nc.scalar.activation(h1[:, m, t2s], ph[:], AF.Gelu,
                                             bias=b1_t[:, m:m + 1], scale=1.0)
                for mo in range(DK):
                    # W2 streamed in two 16-ktile halves (SBUF pressure)
                    w2h = []
                    for hf in range(2):
                        wt2 = wp.tile([128, 16, 128], BF, tag=f"w2_{hf}")
                        (nc.scalar if hf == 0 else nc.gpsimd).dma_start(
                            wt2[:], w2_t[mo, :, ds(hf * 16, 16)])
                        w2h.append(wt2)
                    for t2 in range(2):
                        pv = psums.tile([128, 512], F32, tag="mm")
                        for ks in range(32):
                            nc.tensor.matmul(pv[:], w2h[ks // 16][:, ks % 16, :],
                                             h1[:, ks, ds(t2 * 512, 512)],
                                             start=(ks == 0), stop=(ks == 31))
                        out_cb(mo, t2, pv)

            def pff_out(mo, t2, pv):
                t2s = ds(t2 * 512, 512)
                v2t = strm.tile([128, 512], F32, tag="v2t")
                nc.vector.scalar_tensor_tensor(v2t[:], pv[:], pb2_t[:, mo:mo + 1],
                                               y0buf[:, mo, t2s], op0=OP.add,
                                               op1=OP.add)
                xt = wp.tile([128, 512], MMDT, tag="xt")
                nc.gpsimd.dma_start(xt[:], xTc[t2, :, mo, :])
                t3 = strm.tile([128, 512], F32, tag="t3")
                nc.vector.tensor_mul(t3[:], v2t[:], ubuf[:, mo, t2s])
                nc.vector.tensor_add(x1buf[:, mo, t2s], t3[:], xt[:])

            ffn_phase(lambda t2: (lambda kk, s=ds(t2 * 512, 512):
                                  y0buf[:, kk, s]),
                      pW1t, pb1_t, pW2t, pff_out, nm="4")

            def ffn2_out(mo, t2, pv):
                ot = strm.tile([128, 512], F32, tag="ot")
                nc.vector.scalar_tensor_tensor(ot[:], pv[:], bf2_t[:, mo:mo + 1],
                                               x1buf[:, mo, ds(t2 * 512, 512)],
                                               op0=OP.add, op1=OP.add)
                nc.sync.dma_start(outT[ts(mo, 128), ds(t2 * 512, 512)], ot[:])

            ffn_phase(lambda t2: (lambda kk, s=ds(t2 * 512, 512):
                                  x1buf[:, kk, s]),
                      Wf1t, bf1_t, Wf2t, ffn2_out, nm="5")

    nc.compile()
    return nc


_NC_CACHE = {}


def _get_nc(debug=False):
    key = ("dbg" if debug else "nc")
    if key not in _NC_CACHE:
        _NC_CACHE[key] = build_nc(debug)
    return _NC_CACHE[key]


def _pack_w(w, bf, part_tiles):
    """W[K, N] -> [N/128, 128, K/128, 128] (m, p, kk, n) contiguous tiles."""
    import ml_dtypes
    K, N = w.shape
    arr = np.asarray(w, np.float32).reshape(K // 128, 128, N // 128, 128)
    arr = arr.transpose(2, 1, 0, 3)
    return np.ascontiguousarray(arr.astype(ml_dtypes.bfloat16) if bf else arr)


def make_in_maps(inputs):
    import ml_dtypes
    x = np.asarray(inputs["x"], dtype=np.float32)
    projTdn = np.ascontiguousarray(
        (np.asarray(inputs["proj_mat"], np.float32).T * DN).astype(ml_dtypes.bfloat16))

    f32 = lambda k: np.asarray(inputs[k], np.float32)
    # Wot2[m, p, h, j] = Wo[h*64+p, m*128+j]
    wot2 = f32("Wo").reshape(H, 64, DK, 128).transpose(2, 1, 0, 3)
    common = dict(
        projTdn=projTdn,
        Wqt=_pack_w(f32("Wq"), True, 4),
        Wkt=_pack_w(f32("Wk"), True, 4),
        Wvt=np.ascontiguousarray(
            f32("Wv").reshape(DK, 128, INNER).transpose(1, 0, 2)
            .astype(ml_dtypes.bfloat16)),
        Wot2=np.ascontiguousarray(wot2.astype(ml_dtypes.bfloat16)),
        projWt=_pack_w(f32("proj_W"), True, 8),
        pW1t=_pack_w(f32("pW1"), True, 32),
        pW2t=_pack_w(f32("pW2"), True, 8),
        Wf1t=_pack_w(f32("Wf1"), True, 32),
        Wf2t=_pack_w(f32("Wf2"), True, 8),
        bv_row=np.ascontiguousarray(f32("bv").reshape(1, INNER)),
    )
    vecs = np.zeros((128, VEC_COLS), np.float32)
    for name, (off, n) in VEC_PACK.items():
        vecs[:, off:off + n] = f32(name).reshape(n, 128).T
    vecs[10, 104] = np.log(EPSK)
    common["vecs"] = np.ascontiguousarray(vecs)

    in_maps = []
    for c in range(N_CORES):
        b, off = c // 2, (c % 2) * TM
        x_rot = np.roll(x[b], -off, axis=0)            # my tokens first
        m = dict(common)
        # xTc[c, p, kk, ti] = x_rot[c*512+ti, kk*128+p]
        m["xTc"] = np.ascontiguousarray(
            x_rot.reshape(4, 512, DK, 128).transpose(0, 3, 2, 1))
        in_maps.append(m)
    return in_maps


def _run(inputs, trace=False, debug=False):
    nc = _get_nc(debug)
    in_maps = make_in_maps(inputs)
    res = run_bass_kernel_spmd(nc, in_maps, core_ids=list(range(N_CORES)),
                               trace=trace)
    x = np.asarray(inputs["x"], dtype=np.float32)
    out = np.empty_like(x)
    for c in range(N_CORES):
        b, off = c // 2, (c % 2) * TM
        out[b, off:off + TM] = res.results[c]["outT"].T
    return out, res


def kernel(**inputs):
    out, _ = _run(inputs, trace=False)
    return out
